# revision 3
# baseline (speedup 1.0000x reference)
"""LSH similarity-matrix kernel for Trainium2 (8 NeuronCores, data-parallel over batch).

Math: reference computes, per (l, b):
    c1 = (query_embed @ r.T > 0),  c2 = (doc_embed @ r.T > 0)   in {0,1}
    ham = s1 + s2 - 2*c1@c2.T ;  sim = cos(pi/NB * ham), masked where tok==0.
With +-1 codes U = 2c-1 and S = U1 @ U2.T:  ham = (NB - S)/2, so
    sim = sin(pi/(2*NB) * S).
Masks fold into the embeddings: a zeroed embedding row projects to 0,
sign(0) = 0 gives a zero code row, so S = 0 and sin(0) = 0 — exactly the
masked output. Masked doc tokens are gathered away host-side entirely
(output columns scattered back as zeros); masked query rows are zeroed.

Precision: PE fp32 matmul runs at 4 cycles/row, but float32r (TF32,
11-bit mantissa) runs at 1 cycle/row for moving dims >= 256. A single
TF32 projection flips ~1.5k hash bits (sim absmax ~9e-3), so the
projection uses the exact 3-term compensated split
    proj = rh@eh + rh@el + rl@eh,   xh = tf32(x), xl = tf32(x - xh)
which carries ~22 mantissa bits per operand and reproduces fp32 signs
(0 flips on the benchmark data; verified on hardware). The code dot is
exact in bf16 (+-1 values, fp32 PSUM accumulation).

r is pre-scaled by 2^66 host-side so the DVE sign alternative
clamp(x, -1, 1) = max(min(x,1),-1) is exact (any |proj| > 2^-66 maps to
+-1); sign work is split between the ACT (Sign activation) and DVE
(clamp tensor_scalar) engines.
"""
import os
import sys

sys.path.insert(0, "/opt/trn_rl_repo")

from contextlib import ExitStack

import numpy as np

import concourse.bass as bass
import concourse.mybir as mybir
import concourse.tile as tile
from concourse import bacc
from concourse.bass_utils import run_bass_kernel_spmd

L, BAT, A, BDOC, D, NB = 2, 32, 64, 1024, 128, 1024
CORES = 8
BPC = BAT // CORES          # batches per core
JOBS = BPC * L              # (b, l) pairs per core
CH = NB // 128              # 8 bit-chunks
SCALE = float(2.0 ** 66)
PI = float(np.pi)
DVE_SIGN_CHUNKS = 3         # doc-sign chunks k < this go to DVE, rest to ACT

F32 = mybir.dt.float32
F32R = mybir.dt.float32r
BF16 = mybir.dt.bfloat16
Alu = mybir.AluOpType
Act = mybir.ActivationFunctionType

_BUILD_CACHE: dict = {}


def _col_splits(n):
    """Split [0, n) into PSUM-bank-sized pieces (<=512), each >=256 where
    possible (float32r matmuls drop to 1/4 rate below 256 moving columns)."""
    out = []
    c0 = 0
    rem = n
    while rem > 0:
        if rem > 512:
            # keep the tail >= 256: if the remainder after a 512 piece would
            # be < 256, balance the last two pieces instead
            if rem - 512 < 256:
                a = (rem // 2 + 63) // 64 * 64
                out.append((c0, c0 + a))
                out.append((c0 + a, c0 + rem))
                break
            take = 512
        else:
            take = rem
        out.append((c0, c0 + take))
        c0 += take
        rem -= take
    return out


def _build(pad_c: int, pad_t: int):
    """Per-core SPMD program. pad_t: doc tokens DMA'd/transposed (mult of
    128); pad_c: doc tokens projected/dotted (mult of 64, <= pad_t)."""
    ntile = pad_t // 128
    splits = _col_splits(pad_c)
    nc = bacc.Bacc("TRN2", target_bir_lowering=False, debug=False)

    QE = nc.dram_tensor("qe", [BPC, L, A, D], F32, kind="ExternalInput").ap()
    DE = nc.dram_tensor("de", [BPC, L, pad_t, D], F32, kind="ExternalInput").ap()
    RT = nc.dram_tensor("rt", [D, NB], F32, kind="ExternalInput").ap()
    IDT = nc.dram_tensor("ident", [D, D], F32, kind="ExternalInput").ap()
    OUT = nc.dram_tensor("out", [BPC, L, A, pad_c], F32, kind="ExternalOutput").ap()

    with tile.TileContext(nc) as tc, ExitStack() as ctx:
        const = ctx.enter_context(tc.tile_pool(name="const", bufs=1))
        jobp = ctx.enter_context(tc.tile_pool(name="jobp", bufs=2))
        outp = ctx.enter_context(tc.tile_pool(name="outp", bufs=2))
        ps_t = ctx.enter_context(tc.tile_pool(name="ps_t", bufs=1, space="PSUM"))
        ps_p = ctx.enter_context(tc.tile_pool(name="ps_p", bufs=2, space="PSUM"))
        ps_s = ctx.enter_context(tc.tile_pool(name="ps_s", bufs=1, space="PSUM"))

        # ---- constants ----
        rt_raw = const.tile([D, NB], F32, tag="rt_raw", name="rt_raw")
        nc.sync.dma_start(out=rt_raw, in_=RT)
        ident = const.tile([D, D], F32, tag="ident", name="ident")
        nc.sync.dma_start(out=ident, in_=IDT)
        # rh | rl, rounded on-chip (DVE writes float32r = TF32)
        rhl = const.tile([D, 2 * NB], F32R, tag="rhl", name="rhl")
        nc.vector.tensor_copy(rhl[:, 0:NB], rt_raw)
        nc.vector.tensor_tensor(rhl[:, NB:2 * NB], rt_raw, rhl[:, 0:NB],
                                Alu.subtract)

        # ---- query phase: transpose, split, project, sign ----
        qnat = const.tile([D, BPC * D], F32, tag="qnat", name="qnat")
        for b in range(BPC):
            nc.sync.dma_start(
                out=qnat[:, b * 128:(b + 1) * 128],
                in_=QE[b].rearrange("l t d -> (l t) d"),
            )
        qps = ps_t.tile([D, pad_t], F32, tag="tps", name="qps")[:, 0:BPC * D]
        for b in range(BPC):
            nc.tensor.transpose(qps[:, b * 128:(b + 1) * 128],
                                qnat[:, b * 128:(b + 1) * 128], ident)
        qh = const.tile([D, BPC * D], F32R, tag="qh", name="qh")
        nc.vector.tensor_copy(qh, qps)
        ql = const.tile([D, BPC * D], F32R, tag="ql", name="ql")
        nc.vector.tensor_tensor(ql, qps, qh, Alu.subtract)

        U1 = const.tile([D, CH * BPC * D], BF16, tag="U1", name="U1")
        for k in range(CH):
            rh_k = rhl[:, k * 128:(k + 1) * 128]
            rl_k = rhl[:, NB + k * 128:NB + (k + 1) * 128]
            qp = ps_p.tile([D, pad_c], F32, tag="pp", name=f"qp{k}")[:, 0:BPC * D]
            nc.tensor.matmul(qp, rh_k, qh, start=True, stop=False)
            nc.tensor.matmul(qp, rh_k, ql, start=False, stop=False)
            nc.tensor.matmul(qp, rl_k, qh, start=False, stop=True)
            u1k = U1[:, k * BPC * D:(k + 1) * BPC * D]
            if k % 2 == 0:
                nc.vector.tensor_scalar(u1k, qp, 1.0, -1.0, Alu.min, Alu.max)
            else:
                nc.scalar.activation(u1k, qp, Act.Sign)

        # ---- doc jobs ----
        for b in range(BPC):
            for l in range(L):
                dnat = jobp.tile([D, ntile * D], F32, tag="dnat",
                                 name=f"dnat{b}{l}")
                nc.sync.dma_start(
                    out=dnat[:].rearrange("p (tc d) -> p tc d", d=128),
                    in_=DE[b, l].rearrange("(tc tp) d -> tp tc d", tp=128),
                )
                dT = ps_t.tile([D, pad_t], F32, tag="tps", name=f"dT{b}{l}")
                for t in range(ntile):
                    nc.tensor.transpose(dT[:, t * 128:(t + 1) * 128],
                                        dnat[:, t * 128:(t + 1) * 128], ident)
                eh = jobp.tile([D, pad_t], F32R, tag="eh", name=f"eh{b}{l}")
                nc.vector.tensor_copy(eh, dT)
                el = jobp.tile([D, pad_t], F32R, tag="el", name=f"el{b}{l}")
                nc.vector.tensor_tensor(el, dT, eh, Alu.subtract)

                U2 = jobp.tile([D, CH * pad_c], BF16, tag="U2", name=f"U2{b}{l}")
                for k in range(CH):
                    rh_k = rhl[:, k * 128:(k + 1) * 128]
                    rl_k = rhl[:, NB + k * 128:NB + (k + 1) * 128]
                    pp = ps_p.tile([D, pad_c], F32, tag="pp", name=f"pp{b}{l}{k}")
                    # stationary-grouped order: rh (4 MMs), then rl (2 MMs)
                    for c0, c1 in splits:
                        nc.tensor.matmul(pp[:, c0:c1], rh_k, eh[:, c0:c1],
                                         start=True, stop=False)
                    for c0, c1 in splits:
                        nc.tensor.matmul(pp[:, c0:c1], rh_k, el[:, c0:c1],
                                         start=False, stop=False)
                    for c0, c1 in splits:
                        nc.tensor.matmul(pp[:, c0:c1], rl_k, eh[:, c0:c1],
                                         start=False, stop=True)
                    u2k = U2[:, k * pad_c:(k + 1) * pad_c]
                    if k < DVE_SIGN_CHUNKS:
                        nc.vector.tensor_scalar(u2k, pp, 1.0, -1.0,
                                                Alu.min, Alu.max)
                    else:
                        nc.scalar.activation(u2k, pp, Act.Sign)

                # code dot: S[a, c] = sum_k U1[k, a] * U2[k, c]
                S = ps_s.tile([A, pad_c], F32, tag="S", name=f"S{b}{l}")
                qcol = b * 128 + l * 64
                for c0, c1 in splits:
                    for k in range(CH):
                        nc.tensor.matmul(
                            S[:, c0:c1],
                            U1[:, k * BPC * D + qcol:k * BPC * D + qcol + 64],
                            U2[:, k * pad_c + c0:k * pad_c + c1],
                            start=(k == 0), stop=(k == CH - 1),
                        )
                sim = outp.tile([A, pad_c], F32, tag="sim", name=f"sim{b}{l}")
                nc.scalar.activation(sim, S, Act.Sin, scale=PI / (2.0 * NB))
                nc.sync.dma_start(out=OUT[b, l], in_=sim)

    nc.compile()
    return nc


def kernel(query_embed, doc_embed, query_tok, doc_tok, r):
    query_embed = np.ascontiguousarray(query_embed, dtype=np.float32)
    doc_embed = np.ascontiguousarray(doc_embed, dtype=np.float32)
    r = np.ascontiguousarray(r, dtype=np.float32)

    qmask = (np.asarray(query_tok) != 0)
    dmask = (np.asarray(doc_tok) != 0)

    # active doc tokens, padded: pad_c (compute) mult of 64, pad_t (dma/
    # transpose) mult of 128
    counts = dmask.sum(axis=1)
    pad_c = max(128, int(-(-int(counts.max()) // 64) * 64))
    pad_c = min(pad_c, BDOC)
    pad_t = min(BDOC, int(-(-pad_c // 128) * 128))

    # host staging
    qe_m = query_embed * qmask[None, :, :, None].astype(np.float32)
    rt = np.ascontiguousarray(r.T * SCALE)
    ident = np.eye(D, dtype=np.float32)

    idxs = [np.flatnonzero(dmask[g]) for g in range(BAT)]
    in_maps = []
    for c in range(CORES):
        b0 = c * BPC
        qe_c = np.ascontiguousarray(
            qe_m[:, b0:b0 + BPC].transpose(1, 0, 2, 3))  # [BPC, L, A, D]
        de_c = np.zeros((BPC, L, pad_t, D), dtype=np.float32)
        for b in range(BPC):
            idx = idxs[b0 + b]
            de_c[b, :, :len(idx)] = doc_embed[:, b0 + b, idx]
        in_maps.append({"qe": qe_c, "de": de_c, "rt": rt, "ident": ident})

    key = (pad_c, pad_t)
    if key not in _BUILD_CACHE:
        _BUILD_CACHE[key] = _build(pad_c, pad_t)
    nc = _BUILD_CACHE[key]

    res = run_bass_kernel_spmd(nc, in_maps, core_ids=list(range(CORES)))

    out = np.zeros((BAT, L, A, BDOC), dtype=np.float32)
    for c in range(CORES):
        o_c = res.results[c]["out"]  # [BPC, L, A, pad_c]
        b0 = c * BPC
        for b in range(BPC):
            idx = idxs[b0 + b]
            for li in range(L):
                out[b0 + b, li][:, idx] = o_c[b, li, :, :len(idx)]
    return out
